# revision 2
# baseline (speedup 1.0000x reference)
"""Trainium2 Bass kernel v5 for nn_GateCircuit.

out = sigmoid(cos(x @ W[0]) * cos(params[0])), data-parallel over 8 cores.

vs v4:
- x shipped as float16 (half the HBM traffic -> x halves land ~2.5us
  earlier at the ~179GB/s per-core cap; fp16 dots run in DVE 2x mode).
  fp16 keeps 11 mantissa bits: z error ~0.01 rad, final rel err ~3e-3.
- wp packed into one fp16 tensor: cols 0:514 carry the raw f32 bits of
  [W[0], params[0]] (Pool chain bitcasts them back to f32), cols 514:771
  the fp16 row for a fast single-pass fp16 PE broadcast.
- no ACT engine use at all: the sigmoid is a DVE cubic
  (max abs err 1.7e-4 on [-1,1]); with no activation-table loads the
  scalar ring is free, so wp lands at ~8.6us instead of ~10.5us and
  table data stops competing with x for HBM.
"""

import math
import os

import numpy as np

_NCORES = 8
_B = 4096
_F = 256
_BS = _B // _NCORES
_NT = _BS // 128
_INV_TWO_PI = float(1.0 / (2.0 * math.pi))

_C0 = 0.9985678609910458
_C1 = -19.552759014070162
_C2 = 61.10740166704636
_C3 = -59.580321884808846
_R2 = _C2 / _C3
_R1 = _C1 / _C3
_MAGIC = 12582912.0
_SA = 0.24923215581449706    # sigmoid(y) ~= 0.5 + SA*y + SB*y^3 on [-1,1]
_SB = -0.018173673837602548

_CACHE: dict = {}


def _build():
    import contextlib

    import concourse.bacc as bacc
    import concourse.bass as bass
    import concourse.mybir as mybir

    f32 = mybir.dt.float32
    f16 = mybir.dt.float16
    Alu = mybir.AluOpType

    no_memset_patch = bool(os.environ.get("BASSK_NO_MEMSET_PATCH"))

    @contextlib.contextmanager
    def patched_const_memsets():
        if no_memset_patch:
            yield
            return
        cls = bass.BassGpSimd
        orig = cls.memset

        class _FakeInst:
            def then_inc(self, *a, **k):
                return self

        def fake_memset(self, ap, constant):
            return _FakeInst()

        cls.memset = fake_memset
        try:
            yield
        finally:
            cls.memset = orig

    with patched_const_memsets():
        nc = bacc.Bacc("TRN2", target_bir_lowering=False, debug=False,
                       num_devices=_NCORES)

    x_d = nc.dram_tensor("x", [_BS, _F], f16, kind="ExternalInput")
    wp_d = nc.dram_tensor("wp", [1, 772], f16,
                          kind="ExternalInput")  # f32 bits + f16 row + pad
    o_d = nc.dram_tensor("o", [_BS], f32, kind="ExternalOutput")

    es = contextlib.ExitStack()

    def sb(name, shape, dt=f32):
        return es.enter_context(nc.sbuf_tensor(name, shape, dt))

    ps = lambda name, shape: es.enter_context(nc.psum_tensor(name, shape, f32))
    sem = lambda name: es.enter_context(nc.semaphore(name))

    xt = sb("xt", [128, _NT * _F], f16)      # 2KB/partition
    wpt = sb("wpt", [1, 772], f16)
    ones_h = sb("ones_h", [1, 128], f16)
    ones_f = sb("ones_f", [1, 128], f32)
    chain = sb("chain", [1, 6])
    scb2 = sb("scb2", [1, 2])
    w_h = sb("w_h", [128, _F], f16)
    z = sb("z", [128, _NT])
    km = sb("km", [128, _NT])
    f_t = sb("f_t", [128, _NT])
    v_t = sb("v_t", [128, _NT])
    g_t = sb("g_t", [128, _NT])
    q3 = sb("q3", [128, _NT])
    y_t = sb("y_t", [128, _NT])
    y2 = sb("y2", [128, _NT])
    tc = sb("tc", [128, _NT])
    mc = sb("mc", [128, _NT])
    ot = sb("ot", [128, _NT])

    wps = ps("wps", [128, _F + 1])
    scbps = ps("scbps", [128, 2])

    s_wp = sem("s_wp")
    s_ones = sem("s_ones")
    s_bc1 = sem("s_bc1")
    s_whc = sem("s_whc")
    s_scb2 = sem("s_scb2")
    s_bcscb = sem("s_bcscb")
    s_zv = sem("s_zv")
    s_pc = sem("s_pc")
    s_pp = sem("s_pp")
    s_sig = sem("s_sig")
    s_out = sem("s_out")

    xr = x_d.ap().rearrange("(p n) f -> p (n f)", n=_NT)  # [128, 1024] f16
    wpt_f32 = wpt[0:1, 0:2 * (_F + 1)].bitcast(f32)       # [1, 257] f32 view
    wpt_h = wpt[0:1, 514:771]                              # [1, 257] f16 row

    # ---- sync: x halves ----
    split = [int(v) for v in
             os.environ.get("BASSK_XSPLIT", "512,512").split(",")]
    assert sum(split) == _NT * _F
    col = 0
    xdma_sems = []
    for w_cols in split:
        s_d = sem(f"s_xd{col}")
        nc.sync.dma_start(xt[:, col:col + w_cols],
                          xr[:, col:col + w_cols]).then_inc(s_d, 16)
        col += w_cols
        xdma_sems.append((s_d, col))

    # ---- scalar: just the wp DMA (HWDGE ring free of table loads) ----
    nc.scalar.dma_start(wpt[:, :], wp_d[:, :]).then_inc(s_wp, 16)

    # ---- Pool: ones; params chain [1,1] on the f32 view, linked ----
    nc.gpsimd.memset(ones_h[:, :], 1.0).then_inc(s_ones, 1)
    nc.gpsimd.memset(ones_f[:, :], 1.0).then_inc(s_ones, 1)
    nc.gpsimd.wait_ge(s_wp, 16)
    c = chain
    p0 = wpt_f32[0:1, _F:_F + 1]
    nc.gpsimd.tensor_scalar(c[:, 0:1], p0, _INV_TWO_PI, 1.0,
                            op0=Alu.mult, op1=Alu.mult).then_inc(s_pc, 1)
    nc.gpsimd.wait_ge(s_pc, 1)
    nc.gpsimd.tensor_tensor(c[:, 1:2], c[:, 0:1], c[:, 0:1],
                            op=Alu.mult).then_inc(s_pc, 1)
    nc.gpsimd.wait_ge(s_pc, 2)
    nc.gpsimd.tensor_scalar(c[:, 2:3], c[:, 1:2], _R2, 1.0,
                            op0=Alu.add, op1=Alu.mult).then_inc(s_pc, 1)
    nc.gpsimd.wait_ge(s_pc, 3)
    nc.gpsimd.tensor_tensor(c[:, 3:4], c[:, 2:3], c[:, 1:2],
                            op=Alu.mult).then_inc(s_pc, 1)
    nc.gpsimd.wait_ge(s_pc, 4)
    nc.gpsimd.tensor_scalar(c[:, 4:5], c[:, 3:4], _R1, 1.0,
                            op0=Alu.add, op1=Alu.mult).then_inc(s_pc, 1)
    nc.gpsimd.wait_ge(s_pc, 5)
    nc.gpsimd.tensor_tensor(c[:, 5:6], c[:, 4:5], c[:, 1:2],
                            op=Alu.mult).then_inc(s_pc, 1)
    nc.gpsimd.wait_ge(s_pc, 6)
    nc.gpsimd.tensor_scalar(scb2[:, 0:1], c[:, 5:6], _C3 * _C3, _C3 * _C0,
                            op0=Alu.mult, op1=Alu.add)
    nc.gpsimd.tensor_scalar(scb2[:, 1:2], c[:, 5:6], _C3 * _C0, _C0 * _C0,
                            op0=Alu.mult, op1=Alu.add).then_inc(s_scb2, 1)

    # ---- PE: fp16 broadcast of w row; fp32 broadcast of [scale,bias] ----
    nc.tensor.wait_ge(s_ones, 2)
    nc.tensor.wait_ge(s_wp, 16)
    nc.tensor.matmul(wps[:, :], ones_h[:, :], wpt_h).then_inc(s_bc1, 1)
    nc.tensor.wait_ge(s_scb2, 1)
    nc.tensor.matmul(scbps[:, :], ones_f[:, :], scb2[:, :]).then_inc(s_bcscb, 1)

    # ---- DVE: w fp16 copy; 4 fp16 dots (2x mode); poly; cubic sigmoid ----
    nc.vector.wait_ge(s_bc1, 1)
    nc.vector.tensor_scalar(w_h[:, :], wps[:, 0:_F], 1.0, 0.0,
                            op0=Alu.mult, op1=Alu.add).then_inc(s_whc, 1)
    nc.vector.wait_ge(s_whc, 1)
    waited = set()
    for n in range(_NT):
        prod = sb(f"prodv{n}", [128, _F], f16)
        for idx, (s_d, end_col) in enumerate(xdma_sems):
            if end_col >= (n + 1) * _F:
                if idx not in waited:
                    nc.vector.wait_ge(s_d, 16)
                    waited.add(idx)
                break
        nc.vector.scalar_tensor_tensor(
            prod[:, :], xt[:, n * _F:(n + 1) * _F], _INV_TWO_PI, w_h[:, :],
            op0=Alu.mult, op1=Alu.mult,
            accum_out=z[:, n:n + 1],
        ).then_inc(s_zv, 1)
    nc.vector.wait_ge(s_zv, _NT)
    nc.vector.tensor_scalar(km[:, :], z[:, :], _MAGIC, 1.0,
                            op0=Alu.add, op1=Alu.mult).then_inc(s_pp, 1)
    nc.vector.wait_ge(s_pp, 1)
    nc.vector.scalar_tensor_tensor(f_t[:, :], km[:, :], -_MAGIC, z[:, :],
                                   op0=Alu.add,
                                   op1=Alu.subtract).then_inc(s_pp, 1)
    nc.vector.wait_ge(s_pp, 2)
    nc.vector.tensor_tensor(v_t[:, :], f_t[:, :], f_t[:, :],
                            op=Alu.mult).then_inc(s_pp, 1)
    nc.vector.wait_ge(s_pp, 3)
    nc.vector.scalar_tensor_tensor(g_t[:, :], v_t[:, :], _R2, v_t[:, :],
                                   op0=Alu.add,
                                   op1=Alu.mult).then_inc(s_pp, 1)
    nc.vector.wait_ge(s_pp, 4)
    nc.vector.scalar_tensor_tensor(q3[:, :], g_t[:, :], _R1, v_t[:, :],
                                   op0=Alu.add,
                                   op1=Alu.mult).then_inc(s_pp, 1)
    # y = scale*q3 + bias  (per-partition scalars from PSUM broadcast)
    nc.vector.wait_ge(s_pp, 5)
    nc.vector.wait_ge(s_bcscb, 1)
    nc.vector.tensor_scalar(y_t[:, :], q3[:, :], scbps[:, 0:1],
                            scbps[:, 1:2], op0=Alu.mult,
                            op1=Alu.add).then_inc(s_pp, 1)
    # sigmoid(y) ~= 0.5 + SA*y + SB*y^3 = 0.5 + y*(SA + SB*y^2)
    nc.vector.wait_ge(s_pp, 6)
    nc.vector.tensor_tensor(y2[:, :], y_t[:, :], y_t[:, :],
                            op=Alu.mult).then_inc(s_pp, 1)
    nc.vector.wait_ge(s_pp, 7)
    nc.vector.tensor_scalar(tc[:, :], y2[:, :], _SB, _SA,
                            op0=Alu.mult, op1=Alu.add).then_inc(s_pp, 1)
    nc.vector.wait_ge(s_pp, 8)
    nc.vector.tensor_tensor(mc[:, :], y_t[:, :], tc[:, :],
                            op=Alu.mult).then_inc(s_pp, 1)
    nc.vector.wait_ge(s_pp, 9)
    nc.vector.tensor_scalar(ot[:, :], mc[:, :], 0.5, 1.0,
                            op0=Alu.add, op1=Alu.mult).then_inc(s_sig, 1)

    # ---- output ----
    orr = o_d.ap().rearrange("(p n) -> p n", n=_NT)
    nc.sync.wait_ge(s_sig, 1)
    nc.sync.dma_start(orr[:, :], ot[:, :]).then_inc(s_out, 16)

    es.close()
    nc.compile()
    return nc


def _get_nc():
    if "nc" not in _CACHE:
        _CACHE["nc"] = _build()
    return _CACHE["nc"]


def _in_maps(x, W, params):
    x16 = np.ascontiguousarray(np.asarray(x, dtype=np.float32)
                               .astype(np.float16))
    W = np.asarray(W, dtype=np.float32)
    params = np.asarray(params, dtype=np.float32)
    wp_row = np.concatenate([W[0], params[0:1]]).astype(np.float32)  # [257]
    wp_bits = wp_row.view(np.float16)                                # [514]
    wp_h = wp_row.astype(np.float16)                                 # [257]
    wp_all = np.concatenate(
        [wp_bits, wp_h, np.zeros(1, np.float16)]).reshape(1, 772)
    wp_all = np.ascontiguousarray(wp_all)
    return [
        {"x": x16[c * _BS:(c + 1) * _BS], "wp": wp_all}
        for c in range(_NCORES)
    ]


def run_spmd(x, W, params, **kw):
    import time

    from concourse import bass_utils

    nc = _get_nc()
    in_maps = _in_maps(x, W, params)
    last = None
    for attempt in range(4):
        try:
            return bass_utils.run_bass_kernel_spmd(
                nc, in_maps, list(range(_NCORES)), **kw
            )
        except Exception as e:
            last = e
            time.sleep(2.0 * (attempt + 1))
    raise last


def kernel(x, W, params):
    res = run_spmd(x, W, params)
    return np.concatenate([res.results[c]["o"] for c in range(_NCORES)], axis=0)
